# revision 11
# baseline (speedup 1.0000x reference)
"""HashGrid embedding lookup (nn_HashGridPyTorch) as a TRN2 Bass kernel.

Strategy (v2)
-------------
Data-parallel over the point batch: 2^20 points split across 8 NeuronCores.
The gather uses the custom InstDMAGatherAnt GPSIMD ucode (one descriptor per
lookup, ~0.34ns/desc) instead of per-128-lookup indirect DMAs.

Table re-layout (host, once per kernel call):
  - tables -> bf16. Rows grouped into "slots" of R=16 rows (64B each).
  - 4 levels share each 256B-stride table row (dma_gather's index stride must
    be a multiple of 256B); level l lives at column 64*k(l) of group g(l).
  - slot index = min(h, size-1) >> 4  (<= 32767: fits the ucode's int16 idx)
  - in-slot row = min(h, size-1) & 15 (resolved on-chip by a select tree)

Per core, per tile of 128x256 points (4 tiles):
  1. SEL-pipeline (DVE, natural layout [pi, sigma], point f = sigma*128+pi):
     spatial-hash bit-recursion -> sel = min(h,size-1)&15 per level.
  2. IDX-pipeline (DVE, queue-band layout): same recursion; partition
     32q+j16 (+16 dup for the tx Q7 core) holds the wrap-16 int16 idx stream
     of gather-chunk q, so each dma_gather queue reads its indices from its
     own partition band with no on-chip shuffle.
  3. 64 dma_gathers (16 levels x 4 chunks, queue=chunk): each pulls 8192
     64B slots (16 bf16 rows) into SBUF at [f%128, f//128].
  4. Select tree (DVE): 4 rounds of bitwise blend ((lo^hi)&m)^lo on the
     int32 view (one int32 = one bf16 row-pair) picks the in-slot row.
  5. One DMA writes the [128, 256, 16] int32 (bf16-pair) tile out; host
     converts bf16->f32 and un-permutes.
"""

import numpy as np
import ml_dtypes

import concourse.bass as bass
import concourse.bacc as bacc
import concourse.tile as tile
from concourse import mybir
from concourse import ap_utils
from concourse.bass_utils import run_bass_kernel_spmd

# ---------------------------------------------------------------- constants
L = 16
F = 2
LOG2 = 19
MASK = (1 << LOG2) - 1
C = (73856093, 19349663, 83492791)
CM = tuple(c % (1 << LOG2) for c in C)

RES = [16 << l for l in range(L)]
SIZES = [min(1 << LOG2, (r + 1) ** 3) for r in RES]
OFFSETS = np.concatenate([[0], np.cumsum(SIZES)[:-1]]).astype(np.int64)
TOTAL_PARAMS = int(np.sum(SIZES))  # 7131219

R = 16                                   # rows per slot
NSLOT = [(s + R - 1) // R for s in SIZES]
# 4 levels per 256B table row: group g holds levels LEV_GROUPS[g] at
# bf16-column offsets 32*k
LEV_GROUPS = [[3, 4, 5, 6], [7, 8, 9, 10], [11, 12, 13, 14], [15, 2, 1, 0]]
GROUP_ROWS = 1 << 15                     # 32768 slots per group (max NSLOT)
TAB_ROWS = 4 * GROUP_ROWS
LEV_POS = {}
for g, levs in enumerate(LEV_GROUPS):
    for k, l in enumerate(levs):
        LEV_POS[l] = (g * GROUP_ROWS, 32 * k)

B = 1 << 20
N_CORES = 8
B_CORE = B // N_CORES  # 131072

P = 128

f32 = mybir.dt.float32
i32 = mybir.dt.int32
i16 = mybir.dt.int16
bf16 = mybir.dt.bfloat16


def my_dma_gather(gp, out_ap, in_ap, idxs_ap, num_idxs, elem_size, elem_step,
                  queue_num=0):
    """bass.dma_gather minus the (transpose-only) elem_size%256 assert, with
    single_packet=False (required for num_idxs > ~64)."""
    assert idxs_ap.dtype == mybir.dt.int16
    assert in_ap.dtype == out_ap.dtype
    stride_bytes = elem_step * mybir.dt.size(in_ap.dtype)
    assert stride_bytes % 256 == 0 and stride_bytes // 256 < 256
    assert ap_utils.ap_is_contiguous(out_ap.ap[1:])
    assert ap_utils.ap_is_contiguous(idxs_ap.ap[1:])
    assert in_ap.ap[0][0] == elem_step
    assert in_ap.ap[-1][1] == out_ap.ap[-1][1] == elem_size

    _in_ap = gp.lower_ap_dma(in_ap, for_custom_bir_dma=True)
    _idxs_ap = gp.lower_ap(idxs_ap)
    _out_ap = gp.lower_ap(out_ap)
    return gp.add_instruction(
        mybir.InstDMAGatherAnt(
            name=gp.bass.get_next_instruction_name(),
            ins=[*_in_ap, _idxs_ap, gp.lower_val_access(gp.to_reg(num_idxs))],
            outs=[_out_ap],
            transpose=False,
            num_idxs=num_idxs,
            elem_size=elem_size,
            stride_bytes_256=stride_bytes // 256,
            gen_mode=0,
            single_packet=False,
            queue_num=queue_num,
            sbuf_tokens_per_rank=0,
            sbuf_free_dim_per_rank=0,
            sbuf_free_dim_pad_per_rank=0,
            sbuf_byte_offset=0,
        )
    )


# ------------------------------------------------------------ device program
def _emit_hash(nc, work, xv, nv, size_val, post):
    """Shared spatial-hash pipeline on a [P, nv] point layout.

    xv: [P, 3, nv] coord view (planes). post(l, hc) is called with the
    clamped hash hc (int32 tile [P, nv]) for each level l.
    """
    Alu = mybir.AluOpType
    add_c = float(size_val)
    scale_c = float((1 << LOG2) / (2.0 * size_val))

    i15 = []
    for c in range(3):
        X = work.tile([P, nv], f32, tag="X")
        nc.vector.tensor_scalar(X[:], xv[:, c, :], add_c, scale_c, Alu.add, Alu.mult)
        Fi = work.tile([P, nv], i32, tag="Fi")
        nc.vector.tensor_copy(Fi[:], X[:])                # fp32 -> int32 cast
        Ff = work.tile([P, nv], f32, tag="Ff")
        nc.vector.tensor_copy(Ff[:], Fi[:])               # back to fp32
        gt = work.tile([P, nv], f32, tag="gt")
        nc.vector.tensor_tensor(gt[:], Ff[:], X[:], Alu.is_gt)
        nc.vector.tensor_sub(Ff[:], Ff[:], gt[:])         # robust floor
        nc.vector.tensor_scalar(Ff[:], Ff[:], 0.0, float(MASK), Alu.max, Alu.min)
        Ii = work.tile([P, nv], i32, tag=f"I15{c}")
        nc.vector.tensor_copy(Ii[:], Ff[:])               # exact int
        i15.append(Ii)

    h = work.tile([P, nv], i32, tag="h")
    acc = work.tile([P, nv], i32, tag="acc")
    hc = work.tile([P, nv], i32, tag="hc")
    for c in range(3):
        nc.vector.tensor_scalar(acc[:], i15[c][:], 15, None, Alu.logical_shift_right)
        nc.vector.tensor_scalar(acc[:], acc[:], CM[c], None, Alu.mult)
        if c == 0:
            nc.vector.tensor_scalar(h[:], acc[:], MASK, None, Alu.bitwise_and)
        else:
            nc.vector.tensor_scalar(acc[:], acc[:], MASK, None, Alu.bitwise_and)
            nc.vector.tensor_add(h[:], h[:], acc[:])
    nc.vector.tensor_scalar(h[:], h[:], MASK, None, Alu.bitwise_and)

    def clamp_and_post(l):
        if SIZES[l] - 1 < MASK:
            nc.vector.tensor_scalar(hc[:], h[:], int(SIZES[l] - 1), None, Alu.min)
            post(l, hc)
        else:
            post(l, h)

    clamp_and_post(0)
    for l in range(1, L):
        k = 15 - l
        nc.vector.tensor_scalar(h[:], h[:], 2, None, Alu.mult)
        for c in range(3):
            bit = work.tile([P, nv], i32, tag="bit")
            nc.vector.tensor_scalar(
                bit[:], i15[c][:], k, 1, Alu.logical_shift_right, Alu.bitwise_and
            )
            nc.vector.tensor_scalar(acc[:], bit[:], CM[c], None, Alu.mult)
            nc.vector.tensor_add(h[:], h[:], acc[:])
        nc.vector.tensor_scalar(h[:], h[:], MASK, None, Alu.bitwise_and)
        clamp_and_post(l)


def build_program(size_val=1.0, b_core=B_CORE, t=256, sc=64, nq=4,
                  n_devices=N_CORES):
    """t: points/partition/tile; sc: dst slots per gather chunk (num_idxs =
    128*sc); nq: SWDGE queues used (4 on HW, 1 under CoreSim)."""
    Alu = mybir.AluOpType
    npts = P * t                   # points per tile
    ntiles = b_core // npts
    nf = P * sc                    # lookups per gather
    ch = t // sc                   # gather chunks per (tile, level)
    assert ch % nq == 0
    chq = ch // nq                 # chunks handled per queue band
    nvi = chq * (nf // 16)         # idx-pipe values per partition per level

    nc = bacc.Bacc("TRN2", target_bir_lowering=False, debug=False,
                   num_devices=n_devices, num_swdge_queues=nq)
    xs_t = nc.dram_tensor("xs", [ntiles, P, t * 3], f32, kind="ExternalInput")
    xi_t = nc.dram_tensor("xi", [ntiles, P, nvi * 3], f32,
                          kind="ExternalInput")
    tab_t = nc.dram_tensor("tab", [TAB_ROWS, 128], bf16, kind="ExternalInput")
    out_t = nc.dram_tensor("out", [ntiles, P, t * L], i32, kind="ExternalOutput")

    tab_ap = tab_t.ap()

    with tile.TileContext(nc) as tc:
        with (
            tc.tile_pool(name="io", bufs=2) as io,
            tc.tile_pool(name="work", bufs=2) as work,
            tc.tile_pool(name="selp", bufs=1) as selp,
            tc.tile_pool(name="idxp", bufs=2) as idxp,
            tc.tile_pool(name="blkp", bufs=4) as blkp,
            tc.tile_pool(name="outp", bufs=2) as outp,
        ):
            for ti in range(ntiles):
                xs = io.tile([P, t * 3], f32, tag="xs")
                nc.sync.dma_start(out=xs[:], in_=xs_t.ap()[ti])
                xi = io.tile([P, nvi * 3], f32, tag="xi")
                nc.sync.dma_start(out=xi[:], in_=xi_t.ap()[ti])

                # ---- SEL pipeline (natural layout; nv = t)
                selall = selp.tile([P, L, t], i32, tag="selall")

                def sel_post(l, hc):
                    nc.vector.tensor_scalar(
                        selall[:, l, :], hc[:], R - 1, None, Alu.bitwise_and
                    )

                _emit_hash(nc, work, xs[:].rearrange("p (v c) -> p c v", c=3),
                           t, size_val, sel_post)

                # ---- IDX pipeline (queue-band layout; nv = nvi*16)
                idxall = idxp.tile([P, chq, L, nf // 16], i16, tag="idxall")
                idx32 = idxp.tile([P, nvi], i32, tag="idx32")
                iv = idx32[:].rearrange("p (cl v) -> p cl v", cl=chq)

                def idx_post(l, hc):
                    nc.vector.tensor_scalar(idx32[:], hc[:], 4, None,
                                            Alu.logical_shift_right)
                    for cl in range(chq):
                        nc.vector.tensor_copy(idxall[:, cl, l, :], iv[:, cl, :])

                _emit_hash(nc, work, xi[:].rearrange("p (v c) -> p c v", c=3),
                           nvi, size_val, idx_post)

                # ---- gathers + select
                ot = outp.tile([P, t, L], i32, tag="ot")
                for l in range(L):
                    row_off, col_off = LEV_POS[l]
                    in_ap = tab_ap[row_off : row_off + NSLOT[l],
                                   col_off : col_off + 32]
                    for c_ in range(ch):
                        q = c_ % nq
                        cl = c_ // nq
                        blk = blkp.tile([P, sc, R], i32, tag="blk")
                        my_dma_gather(
                            nc.gpsimd,
                            blk[:].bitcast(bf16),
                            in_ap,
                            idxall[:, cl, l, :],
                            num_idxs=nf,
                            elem_size=2 * R,
                            elem_step=128,
                            queue_num=q,
                        )
                        # select tree: 4 rounds of ((lo^hi)&m)^lo on int32
                        sel = selall[:, l, c_ * sc : (c_ + 1) * sc]
                        cur = blk
                        w = R // 2
                        rnd = 3
                        while w >= 1:
                            m = work.tile([P, sc], i32, tag="m")
                            nc.vector.tensor_scalar(
                                m[:], sel, 31 - rnd, 31,
                                Alu.logical_shift_left, Alu.arith_shift_right,
                            )
                            lo = cur[:, :, 0:w]
                            hi = cur[:, :, w : 2 * w]
                            if w > 1:
                                nxt = work.tile([P, sc, w], i32, tag="nxt")
                                dst = nxt[:]
                            else:
                                dst = ot[:, c_ * sc : (c_ + 1) * sc, l]
                                dst = dst.rearrange("p (s o) -> p s o", o=1)
                            mb = m[:].rearrange("p (s o) -> p s o", o=1
                                                ).broadcast_to([P, sc, w])
                            nc.vector.tensor_tensor(dst, lo, hi, Alu.bitwise_xor)
                            nc.vector.tensor_tensor(dst, dst, mb, Alu.bitwise_and)
                            nc.vector.tensor_tensor(dst, dst, lo, Alu.bitwise_xor)
                            cur = nxt if w > 1 else None
                            w //= 2
                            rnd -= 1

                nc.sync.dma_start(
                    out=out_t.ap()[ti],
                    in_=ot[:].rearrange("p t l -> p (t l)"),
                )
    nc.compile()
    return nc


# ------------------------------------------------------------- host wrappers
def pack_tables(tables):
    """f32 [TOTAL_PARAMS, 2] -> bf16 padded slot table [TAB_ROWS, 128]."""
    tb = np.ascontiguousarray(tables, dtype=np.float32).astype(ml_dtypes.bfloat16)
    out = np.zeros((TAB_ROWS, 128), ml_dtypes.bfloat16)
    for l in range(L):
        row_off, col_off = LEV_POS[l]
        rows = tb[OFFSETS[l] : OFFSETS[l] + SIZES[l]]
        padded = np.zeros((NSLOT[l] * R, 2), ml_dtypes.bfloat16)
        padded[: SIZES[l]] = rows
        out[row_off : row_off + NSLOT[l], col_off : col_off + 32] = (
            padded.reshape(NSLOT[l], 32)
        )
    return out


def permute_x(x_core, t=256, sc=64, nq=4):
    """x [b_core, 3] -> (xs [ntiles,P,t*3], xi [ntiles,P,nvi*16*3])."""
    b_core = x_core.shape[0]
    npts = P * t
    ntiles = b_core // npts
    nf = P * sc
    ch = t // sc
    chq = ch // nq
    nvi = chq * (nf // 16)

    xt = x_core.reshape(ntiles, npts, 3)
    # sel layout: f = sigma*128 + pi -> [pi, sigma]
    xs = xt.reshape(ntiles, t, P, 3).transpose(0, 2, 1, 3)
    xs = np.ascontiguousarray(xs.reshape(ntiles, P, t * 3))

    # idx layout: f = ((c_*sc*128)...), partition 32q+j16(+16):
    # value order per lane: (cl, i16); f = nf*(cl*nq+q) + 16*i16 + j16
    xi4 = xt.reshape(ntiles, chq, nq, nf // 16, 16, 3)  # [ti, cl, q, i16, j16, c]
    xi4 = xi4.transpose(0, 2, 4, 1, 3, 5)               # [ti, q, j16, cl, i16, c]
    xi4 = xi4.reshape(ntiles, nq, 16, nvi * 3)
    xi = np.zeros((ntiles, P, nvi * 3), np.float32)
    for q in range(nq):
        xi[:, 32 * q : 32 * q + 16] = xi4[:, q]
        xi[:, 32 * q + 16 : 32 * q + 32] = xi4[:, q]
    # pad band width when nq < 4 leaves partitions unused (already zeros)
    return xs, np.ascontiguousarray(xi)


def unpermute_out(dev_out, t=256):
    """[ntiles, P, t*L] int32 (bf16 pairs) -> [b_core, 32] f32."""
    ntiles = dev_out.shape[0]
    o = dev_out.reshape(ntiles, P, t, L).view(ml_dtypes.bfloat16)
    o = o.reshape(ntiles, P, t, L, 2).astype(np.float32)
    # f = sigma*128 + pi; row m = ti*npts + f
    o = o.transpose(0, 2, 1, 3, 4)  # [ti, sigma, pi, L, 2]
    return np.ascontiguousarray(o.reshape(ntiles * t * P, L * F))


_CACHE = {}


def _get_program(size_val):
    key = float(size_val)
    if key not in _CACHE:
        _CACHE[key] = build_program(key)
    return _CACHE[key]


def prepare_inputs(inputs, tables, t=256, sc=64, nq=4):
    x = np.ascontiguousarray(np.asarray(inputs, dtype=np.float32))
    tabp = pack_tables(tables)
    in_maps = []
    for i in range(N_CORES):
        xs, xi = permute_x(x[i * B_CORE : (i + 1) * B_CORE], t=t, sc=sc, nq=nq)
        in_maps.append({"xs": xs, "xi": xi, "tab": tabp})
    return in_maps


def run(inputs, tables, size, trace=False):
    size_val = float(np.asarray(size))
    nc = _get_program(size_val)
    in_maps = prepare_inputs(inputs, tables)
    res = run_bass_kernel_spmd(nc, in_maps, list(range(N_CORES)), trace=trace)
    outs = [unpermute_out(res.results[i]["out"]) for i in range(N_CORES)]
    return np.concatenate(outs, axis=0), res


def kernel(inputs, tables, size):
    out, _ = run(inputs, tables, size, trace=False)
    return out


# revision 16
# speedup vs baseline: 2.3940x; 2.3940x over previous
"""HashGrid embedding lookup (nn_HashGridPyTorch) as a TRN2 Bass kernel.

Strategy (v2)
-------------
Data-parallel over the point batch: 2^20 points split across 8 NeuronCores.
The gather uses the custom InstDMAGatherAnt GPSIMD ucode (one descriptor per
lookup, ~0.34ns/desc) instead of per-128-lookup indirect DMAs.

Table re-layout (host, once per kernel call):
  - tables -> bf16. Rows grouped into "slots" of R=16 rows (64B each).
  - 4 levels share each 256B-stride table row (dma_gather's index stride must
    be a multiple of 256B); level l lives at column 64*k(l) of group g(l).
  - slot index = min(h, size-1) >> 4  (<= 32767: fits the ucode's int16 idx)
  - in-slot row = min(h, size-1) & 15 (resolved on-chip by a select tree)

Per core, per tile of 128x256 points (4 tiles):
  1. SEL-pipeline (DVE, natural layout [pi, sigma], point f = sigma*128+pi):
     spatial-hash bit-recursion -> sel = min(h,size-1)&15 per level.
  2. IDX-pipeline (DVE, queue-band layout): same recursion; partition
     32q+j16 (+16 dup for the tx Q7 core) holds the wrap-16 int16 idx stream
     of gather-chunk q, so each dma_gather queue reads its indices from its
     own partition band with no on-chip shuffle.
  3. 64 dma_gathers (16 levels x 4 chunks, queue=chunk): each pulls 8192
     64B slots (16 bf16 rows) into SBUF at [f%128, f//128].
  4. Select tree (DVE): 4 rounds of bitwise blend ((lo^hi)&m)^lo on the
     int32 view (one int32 = one bf16 row-pair) picks the in-slot row.
  5. One DMA writes the [128, 256, 16] int32 (bf16-pair) tile out; host
     converts bf16->f32 and un-permutes.
"""

import numpy as np
import ml_dtypes

import concourse.bass as bass
import concourse.bacc as bacc
import concourse.tile as tile
from concourse import mybir
from concourse import ap_utils
from concourse.bass_utils import run_bass_kernel_spmd

# ---------------------------------------------------------------- constants
L = 16
F = 2
LOG2 = 19
MASK = (1 << LOG2) - 1
C = (73856093, 19349663, 83492791)
CM = tuple(c % (1 << LOG2) for c in C)

RES = [16 << l for l in range(L)]
SIZES = [min(1 << LOG2, (r + 1) ** 3) for r in RES]
OFFSETS = np.concatenate([[0], np.cumsum(SIZES)[:-1]]).astype(np.int64)
TOTAL_PARAMS = int(np.sum(SIZES))  # 7131219

R = 16                                   # rows per slot
NSLOT = [(s + R - 1) // R for s in SIZES]
# 4 levels per 256B table row: group g holds levels LEV_GROUPS[g] at
# bf16-column offsets 32*k
LEV_GROUPS = [[3, 4, 5, 6], [7, 8, 9, 10], [11, 12, 13, 14], [15, 2, 1, 0]]
GROUP_ROWS = 1 << 15                     # 32768 slots per group (max NSLOT)
TAB_ROWS = 4 * GROUP_ROWS
LEV_POS = {}
for g, levs in enumerate(LEV_GROUPS):
    for k, l in enumerate(levs):
        LEV_POS[l] = (g * GROUP_ROWS, 32 * k)

B = 1 << 20
N_CORES = 8
B_CORE = B // N_CORES  # 131072

P = 128

f32 = mybir.dt.float32
i32 = mybir.dt.int32
i16 = mybir.dt.int16
bf16 = mybir.dt.bfloat16


def my_dma_gather(gp, out_ap, in_ap, idxs_ap, num_idxs, elem_size, elem_step,
                  queue_num=0):
    """bass.dma_gather minus the (transpose-only) elem_size%256 assert, with
    single_packet=False (required for num_idxs > ~64)."""
    assert idxs_ap.dtype == mybir.dt.int16
    assert in_ap.dtype == out_ap.dtype
    stride_bytes = elem_step * mybir.dt.size(in_ap.dtype)
    assert stride_bytes % 256 == 0 and stride_bytes // 256 < 256
    assert ap_utils.ap_is_contiguous(out_ap.ap[1:])
    assert ap_utils.ap_is_contiguous(idxs_ap.ap[1:])
    assert in_ap.ap[0][0] == elem_step
    assert in_ap.ap[-1][1] == out_ap.ap[-1][1] == elem_size

    _in_ap = gp.lower_ap_dma(in_ap, for_custom_bir_dma=True)
    _idxs_ap = gp.lower_ap(idxs_ap)
    _out_ap = gp.lower_ap(out_ap)
    return gp.add_instruction(
        mybir.InstDMAGatherAnt(
            name=gp.bass.get_next_instruction_name(),
            ins=[*_in_ap, _idxs_ap, gp.lower_val_access(gp.to_reg(num_idxs))],
            outs=[_out_ap],
            transpose=False,
            num_idxs=num_idxs,
            elem_size=elem_size,
            stride_bytes_256=stride_bytes // 256,
            gen_mode=0,
            single_packet=False,
            queue_num=queue_num,
            sbuf_tokens_per_rank=0,
            sbuf_free_dim_per_rank=0,
            sbuf_free_dim_pad_per_rank=0,
            sbuf_byte_offset=0,
        )
    )


# ------------------------------------------------------------ device program
def _emit_hash(nc, work, xv, nv, size_val, post):
    """Shared spatial-hash pipeline on a [P, nv] point layout.

    xv: [P, 3, nv] coord view (planes). post(l, hc) is called with the
    clamped hash hc (int32 tile [P, nv]) for each level l.
    """
    Alu = mybir.AluOpType
    add_c = float(size_val)
    scale_c = float((1 << LOG2) / (2.0 * size_val))

    i15 = []
    for c in range(3):
        X = work.tile([P, nv], f32, tag="X")
        nc.vector.tensor_scalar(X[:], xv[:, c, :], add_c, scale_c, Alu.add, Alu.mult)
        Fi = work.tile([P, nv], i32, tag="Fi")
        nc.vector.tensor_copy(Fi[:], X[:])                # fp32 -> int32 cast
        Ff = work.tile([P, nv], f32, tag="Ff")
        nc.vector.tensor_copy(Ff[:], Fi[:])               # back to fp32
        gt = work.tile([P, nv], f32, tag="gt")
        nc.vector.tensor_tensor(gt[:], Ff[:], X[:], Alu.is_gt)
        nc.vector.tensor_sub(Ff[:], Ff[:], gt[:])         # robust floor
        nc.vector.tensor_scalar(Ff[:], Ff[:], 0.0, float(MASK), Alu.max, Alu.min)
        Ii = work.tile([P, nv], i32, tag=f"I15{c}")
        nc.vector.tensor_copy(Ii[:], Ff[:])               # exact int
        i15.append(Ii)

    h = work.tile([P, nv], i32, tag="h")
    acc = work.tile([P, nv], i32, tag="acc")
    hc = work.tile([P, nv], i32, tag="hc")
    for c in range(3):
        nc.vector.tensor_scalar(acc[:], i15[c][:], 15, None, Alu.logical_shift_right)
        nc.vector.tensor_scalar(acc[:], acc[:], CM[c], None, Alu.mult)
        if c == 0:
            nc.vector.tensor_scalar(h[:], acc[:], MASK, None, Alu.bitwise_and)
        else:
            nc.vector.tensor_scalar(acc[:], acc[:], MASK, None, Alu.bitwise_and)
            nc.vector.tensor_add(h[:], h[:], acc[:])
    nc.vector.tensor_scalar(h[:], h[:], MASK, None, Alu.bitwise_and)

    def clamp_and_post(l):
        if SIZES[l] - 1 < MASK:
            nc.vector.tensor_scalar(hc[:], h[:], int(SIZES[l] - 1), None, Alu.min)
            post(l, hc)
        else:
            post(l, h)

    clamp_and_post(0)
    for l in range(1, L):
        k = 15 - l
        nc.vector.tensor_scalar(h[:], h[:], 2, None, Alu.mult)
        for c in range(3):
            bit = work.tile([P, nv], i32, tag="bit")
            nc.vector.tensor_scalar(
                bit[:], i15[c][:], k, 1, Alu.logical_shift_right, Alu.bitwise_and
            )
            nc.vector.tensor_scalar(acc[:], bit[:], CM[c], None, Alu.mult)
            nc.vector.tensor_add(h[:], h[:], acc[:])
        nc.vector.tensor_scalar(h[:], h[:], MASK, None, Alu.bitwise_and)
        clamp_and_post(l)


def build_program(size_val=1.0, b_core=B_CORE, t=256, sc=16, nq=4,
                  n_devices=N_CORES):
    """t: points/partition/tile; sc: dst slots per gather chunk (num_idxs =
    128*sc); nq: SWDGE queues used (4 on HW, 1 under CoreSim)."""
    Alu = mybir.AluOpType
    npts = P * t                   # points per tile
    ntiles = b_core // npts
    nf = P * sc                    # lookups per gather
    ch = t // sc                   # gather chunks per (tile, level)
    assert ch % nq == 0
    chq = ch // nq                 # chunks handled per queue band
    nvi = chq * (nf // 16)         # idx-pipe values per partition per level

    nc = bacc.Bacc("TRN2", target_bir_lowering=False, debug=False,
                   num_devices=n_devices, num_swdge_queues=nq)
    xs_t = nc.dram_tensor("xs", [ntiles, P, t * 3], f32, kind="ExternalInput")
    xi_t = nc.dram_tensor("xi", [ntiles, P, nvi * 3], f32,
                          kind="ExternalInput")
    tab_t = nc.dram_tensor("tab", [TAB_ROWS, 128], bf16, kind="ExternalInput")
    out_t = nc.dram_tensor("out", [ntiles, P, t * L], i32, kind="ExternalOutput")

    tab_ap = tab_t.ap()

    with tile.TileContext(nc) as tc:
        with (
            tc.tile_pool(name="io", bufs=2) as io,
            tc.tile_pool(name="work", bufs=2) as work,
            tc.tile_pool(name="selp", bufs=1) as selp,
            tc.tile_pool(name="idxp", bufs=2) as idxp,
            tc.tile_pool(name="blkp", bufs=8) as blkp,
            tc.tile_pool(name="outp", bufs=2) as outp,
        ):
            for ti in range(ntiles):
                xs = io.tile([P, t * 3], f32, tag="xs")
                nc.sync.dma_start(out=xs[:], in_=xs_t.ap()[ti])
                xi = io.tile([P, nvi * 3], f32, tag="xi")
                nc.sync.dma_start(out=xi[:], in_=xi_t.ap()[ti])

                # ---- SEL pipeline (natural layout; nv = t)
                selall = selp.tile([P, L, t], i32, tag="selall")

                def sel_post(l, hc):
                    nc.vector.tensor_scalar(
                        selall[:, l, :], hc[:], R - 1, None, Alu.bitwise_and
                    )

                _emit_hash(nc, work, xs[:].rearrange("p (v c) -> p c v", c=3),
                           t, size_val, sel_post)

                # ---- IDX pipeline (queue-band layout; nv = nvi*16)
                idxall = idxp.tile([P, chq, L, nf // 16], i16, tag="idxall")
                idx32 = idxp.tile([P, nvi], i32, tag="idx32")
                iv = idx32[:].rearrange("p (cl v) -> p cl v", cl=chq)

                def idx_post(l, hc):
                    nc.vector.tensor_scalar(idx32[:], hc[:], 4, None,
                                            Alu.logical_shift_right)
                    for cl in range(chq):
                        nc.vector.tensor_copy(idxall[:, cl, l, :], iv[:, cl, :])

                _emit_hash(nc, work, xi[:].rearrange("p (v c) -> p c v", c=3),
                           nvi, size_val, idx_post)

                # ---- gathers + select
                ot = outp.tile([P, t, L], i32, tag="ot")
                for l in range(L):
                    row_off, col_off = LEV_POS[l]
                    in_ap = tab_ap[row_off : row_off + NSLOT[l],
                                   col_off : col_off + 32]
                    # per-(tile, level) sign-extended bit masks for the
                    # select tree, sliced per chunk below
                    mall = selp.tile([P, 4, t], i32, tag="mall")
                    for r in range(4):
                        nc.vector.tensor_scalar(
                            mall[:, r, :], selall[:, l, :], 31 - r, 31,
                            Alu.logical_shift_left, Alu.arith_shift_right,
                        )
                    for c_ in range(ch):
                        q = c_ % nq
                        cl = c_ // nq
                        blk = blkp.tile([P, sc, R], i32, tag="blk")
                        my_dma_gather(
                            nc.gpsimd,
                            blk[:].bitcast(bf16),
                            in_ap,
                            idxall[:, cl, l, :],
                            num_idxs=nf,
                            elem_size=2 * R,
                            elem_step=128,
                            queue_num=q,
                        )
                        # select tree: 4 rounds of ((lo^hi)&m)^lo on int32
                        cur = blk
                        w = R // 2
                        rnd = 3
                        while w >= 1:
                            m = mall[:, rnd, c_ * sc : (c_ + 1) * sc]
                            lo = cur[:, :, 0:w]
                            hi = cur[:, :, w : 2 * w]
                            if w > 1:
                                nxt = work.tile([P, sc, w], i32, tag="nxt")
                                dst = nxt[:]
                            else:
                                dst = ot[:, c_ * sc : (c_ + 1) * sc, l]
                                dst = dst.rearrange("p (s o) -> p s o", o=1)
                            mb = m.rearrange("p (s o) -> p s o", o=1
                                             ).broadcast_to([P, sc, w])
                            nc.vector.tensor_tensor(dst, lo, hi, Alu.bitwise_xor)
                            nc.vector.tensor_tensor(dst, dst, mb, Alu.bitwise_and)
                            nc.vector.tensor_tensor(dst, dst, lo, Alu.bitwise_xor)
                            cur = nxt if w > 1 else None
                            w //= 2
                            rnd -= 1

                nc.sync.dma_start(
                    out=out_t.ap()[ti],
                    in_=ot[:].rearrange("p t l -> p (t l)"),
                )
    nc.compile()
    return nc


# ------------------------------------------------------------- host wrappers
def pack_tables(tables):
    """f32 [TOTAL_PARAMS, 2] -> bf16 padded slot table [TAB_ROWS, 128]."""
    tb = np.ascontiguousarray(tables, dtype=np.float32).astype(ml_dtypes.bfloat16)
    out = np.zeros((TAB_ROWS, 128), ml_dtypes.bfloat16)
    for l in range(L):
        row_off, col_off = LEV_POS[l]
        rows = tb[OFFSETS[l] : OFFSETS[l] + SIZES[l]]
        padded = np.zeros((NSLOT[l] * R, 2), ml_dtypes.bfloat16)
        padded[: SIZES[l]] = rows
        out[row_off : row_off + NSLOT[l], col_off : col_off + 32] = (
            padded.reshape(NSLOT[l], 32)
        )
    return out


def permute_x(x_core, t=256, sc=16, nq=4):
    """x [b_core, 3] -> (xs [ntiles,P,t*3], xi [ntiles,P,nvi*16*3])."""
    b_core = x_core.shape[0]
    npts = P * t
    ntiles = b_core // npts
    nf = P * sc
    ch = t // sc
    chq = ch // nq
    nvi = chq * (nf // 16)

    xt = x_core.reshape(ntiles, npts, 3)
    # sel layout: f = sigma*128 + pi -> [pi, sigma]
    xs = xt.reshape(ntiles, t, P, 3).transpose(0, 2, 1, 3)
    xs = np.ascontiguousarray(xs.reshape(ntiles, P, t * 3))

    # idx layout: f = ((c_*sc*128)...), partition 32q+j16(+16):
    # value order per lane: (cl, i16); f = nf*(cl*nq+q) + 16*i16 + j16
    xi4 = xt.reshape(ntiles, chq, nq, nf // 16, 16, 3)  # [ti, cl, q, i16, j16, c]
    xi4 = xi4.transpose(0, 2, 4, 1, 3, 5)               # [ti, q, j16, cl, i16, c]
    xi4 = xi4.reshape(ntiles, nq, 16, nvi * 3)
    xi = np.zeros((ntiles, P, nvi * 3), np.float32)
    for q in range(nq):
        xi[:, 32 * q : 32 * q + 16] = xi4[:, q]
        xi[:, 32 * q + 16 : 32 * q + 32] = xi4[:, q]
    # pad band width when nq < 4 leaves partitions unused (already zeros)
    return xs, np.ascontiguousarray(xi)


def unpermute_out(dev_out, t=256):
    """[ntiles, P, t*L] int32 (bf16 pairs) -> [b_core, 32] f32."""
    ntiles = dev_out.shape[0]
    o = dev_out.reshape(ntiles, P, t, L).view(ml_dtypes.bfloat16)
    o = o.reshape(ntiles, P, t, L, 2).astype(np.float32)
    # f = sigma*128 + pi; row m = ti*npts + f
    o = o.transpose(0, 2, 1, 3, 4)  # [ti, sigma, pi, L, 2]
    return np.ascontiguousarray(o.reshape(ntiles * t * P, L * F))


_CACHE = {}


def _get_program(size_val):
    key = float(size_val)
    if key not in _CACHE:
        _CACHE[key] = build_program(key)
    return _CACHE[key]


def prepare_inputs(inputs, tables, t=256, sc=16, nq=4):
    x = np.ascontiguousarray(np.asarray(inputs, dtype=np.float32))
    tabp = pack_tables(tables)
    in_maps = []
    for i in range(N_CORES):
        xs, xi = permute_x(x[i * B_CORE : (i + 1) * B_CORE], t=t, sc=sc, nq=nq)
        in_maps.append({"xs": xs, "xi": xi, "tab": tabp})
    return in_maps


def run(inputs, tables, size, trace=False):
    size_val = float(np.asarray(size))
    nc = _get_program(size_val)
    in_maps = prepare_inputs(inputs, tables)
    res = run_bass_kernel_spmd(nc, in_maps, list(range(N_CORES)), trace=trace)
    outs = [unpermute_out(res.results[i]["out"]) for i in range(N_CORES)]
    return np.concatenate(outs, axis=0), res


def kernel(inputs, tables, size):
    out, _ = run(inputs, tables, size, trace=False)
    return out
